# revision 26
# baseline (speedup 1.0000x reference)
"""MultiHeadEMA Trainium2 Bass kernel.

Reference computation (B=4, S=8192, D=1024, N=2):
    out = silu(conv_causal(x, k) + x * omega)
    k[d, l] = sum_n c[d, n] * q[d, n]^l
    q = 1 - sigmoid(delta) * sigmoid(alpha)
    c = sigmoid(delta) * beta * gamma * sqrt(1/N)

The length-S causal conv with a sum-of-2-exponentials kernel is a pair of
first-order linear recurrences (EMA scans):
    h_n[t] = q_n * h_n[t-1] + x[t]
    y[t]   = c_1 h_1[t] + c_2 h_2[t]
    out[t] = silu(y[t] + omega * x[t])

Sharding: D=1024 split across 8 cores (128 channels each).  Each core works
in [channel-partition, time-free] layout; the scans run on the Vector engine
via TensorTensorScanArith, one recurrence per partition.  The host transposes
x to [B, D, S] while slicing the per-core shards and transposes the per-core
results back while gathering (part of the shard/unshard contract).
"""

import math

import numpy as np

import concourse.bass as bass
import concourse.mybir as mybir
import concourse.tile as tile
from concourse import bacc
from concourse.bass_utils import run_bass_kernel_spmd

B = 4
S = 8192
D = 1024
N_CORES = 8
D_LOC = D // N_CORES  # 128 channels per core
SCALE = math.sqrt(1.0 / 2.0)

F32 = mybir.dt.float32


def build_nc(b=B, d_loc=D_LOC, s=S, t_chunk=2048, act="Silu",
             x_bufs=3, h_bufs=3, tmp_bufs=3, acc_bufs=2):
    """Build the per-core Bass module (SPMD: same NEFF on all cores).

    Inputs (per core):
      x  [b, d_loc, s] f32 — time-major-last shard of the input
      pp [d_loc, 8]    f32 — packed params: q1 q2 c1 c2 w (cols 0-4)
    Output:
      o  [b, d_loc, s] f32
    """
    assert s % t_chunk == 0
    n_chunks = s // t_chunk
    # Non-uniform chunk schedule: small chunks at the very start (fill the
    # pipeline quickly) and at the very end (short drain tail).  Middle runs
    # at full t_chunk.  Only the first/last batch get the ramps.
    def chunk_schedule(bi):
        full = [t_chunk] * n_chunks
        ramp = [t_chunk // 8, t_chunk // 8, t_chunk // 4, t_chunk // 2]
        tail = [t_chunk // 2, t_chunk // 4, t_chunk // 8,
                t_chunk // 16, t_chunk // 16]
        if bi == 0 and n_chunks >= 2:
            return ramp + [t_chunk] * (n_chunks - 1)
        if bi == b - 1 and n_chunks >= 2:
            return [t_chunk] * (n_chunks - 1) + tail
        return full

    nc = bacc.Bacc(
        "TRN2",
        target_bir_lowering=False,
        debug=False,
        enable_asserts=False,
        num_devices=N_CORES,
    )

    x_d = nc.dram_tensor("x", [b, d_loc, s], F32, kind="ExternalInput").ap()
    pp_d = nc.dram_tensor("pp", [d_loc, 12], F32, kind="ExternalInput").ap()
    o_d = nc.dram_tensor("o", [b, d_loc, s], F32, kind="ExternalOutput").ap()

    with tile.TileContext(nc) as tc:
        with (
            tc.tile_pool(name="pp", bufs=1) as pp_pool,
            tc.tile_pool(name="x", bufs=x_bufs) as x_pool,
            tc.tile_pool(name="h", bufs=h_bufs) as h_pool,
            tc.tile_pool(name="tmp", bufs=tmp_bufs) as tmp_pool,
            tc.tile_pool(name="acc", bufs=acc_bufs) as acc_pool,
        ):
            # pp rides the GpSimd SWDGE path so the HWDGE queue's first
            # (cold, ~3us setup) transfer is the first x chunk itself —
            # overlapping the two queue spin-ups at kernel start.
            pp = pp_pool.tile([d_loc, 12], F32, tag="pp")
            nc.gpsimd.dma_start(out=pp[:], in_=pp_d[:])
            # Warm two HWDGE queues with tiny transfers so the first real
            # x-chunk DMA doesn't pay the cold ~3us queue-setup latency.
            warm = pp_pool.tile([d_loc, 2], F32, tag="warm")
            nc.sync.dma_start(out=warm[:, 0:1], in_=x_d[0, :, 0:1])
            nc.sync.dma_start(out=warm[:, 1:2], in_=x_d[0, :, 1:2])
            q1 = pp[:, 0:1]
            q2 = pp[:, 1:2]
            c1 = pp[:, 2:3]
            c2 = pp[:, 3:4]
            w = pp[:, 4:5]
            q1sq_b = pp[:, 5:6].broadcast_to([d_loc, t_chunk // 2])
            q2sq_b = pp[:, 6:7].broadcast_to([d_loc, t_chunk // 2])
            c1q1 = pp[:, 7:8]
            c2q2 = pp[:, 8:9]
            ccw = pp[:, 9:10]

            mult = mybir.AluOpType.mult
            add = mybir.AluOpType.add
            COPY = mybir.ActivationFunctionType.Copy
            ACT = getattr(mybir.ActivationFunctionType, act)

            # Radix-2 polyphase: the time-major scan halves its length by
            # scanning only even positions (h_e[m] = q^2 h_e[m-1] + u[m],
            # u[m] = q*x[2m-1] + x[2m]); odd positions never materialize —
            # they fold into the combine as r_odd = c1q1*h1e + c2q2*h2e +
            # (c1+c2+w)*x_odd.  Strided SBUF access is full-rate on both
            # DVE and ACT (measured), so only the scan shrinks.
            h1_prev = None
            h2_prev = None
            for bi in range(b):
                t0 = 0
                for j, tc_len in enumerate(chunk_schedule(bi)):
                    m = tc_len // 2
                    # x tile with 1-element halo in column 0 (= x[t0-1])
                    xt = x_pool.tile([d_loc, t_chunk + 2], F32, tag="x")
                    if j == 0:
                        nc.vector.memset(xt[:, 0:1], 0.0)
                        nc.sync.dma_start(
                            out=xt[:, 1 : tc_len + 1],
                            in_=x_d[bi, :, t0 : t0 + tc_len],
                        )
                    else:
                        nc.sync.dma_start(
                            out=xt[:, 0 : tc_len + 1],
                            in_=x_d[bi, :, t0 - 1 : t0 + tc_len],
                        )
                    # phase views (columns: [halo, x0, x1, ..., x_{T-1}, pad])
                    xop = xt[:, 0 : 2 * m].rearrange("p (m two) -> p m two", two=2)[:, :, 0]   # x[2m-1]
                    xe = xt[:, 1 : 2 * m + 1].rearrange("p (m two) -> p m two", two=2)[:, :, 0]  # x[2m]
                    xo = xt[:, 2 : 2 * m + 2].rearrange("p (m two) -> p m two", two=2)[:, :, 0]  # x[2m+1]

                    # u_n = q_n * x[2m-1] + x[2m]    (Vector)
                    u1 = acc_pool.tile([d_loc, t_chunk // 2], F32, tag="u1")
                    u2 = acc_pool.tile([d_loc, t_chunk // 2], F32, tag="u2")
                    nc.vector.scalar_tensor_tensor(u1[:, :m], xop, q1, xe, mult, add)
                    nc.vector.scalar_tensor_tensor(u2[:, :m], xop, q2, xe, mult, add)

                    # residual pre-scales on the Scalar engine
                    t1e = tmp_pool.tile([d_loc, t_chunk // 2], F32, tag="t1e")
                    t1o = tmp_pool.tile([d_loc, t_chunk // 2], F32, tag="t1o")
                    nc.scalar.activation(t1e[:, :m], xe, COPY, scale=w)
                    nc.scalar.activation(t1o[:, :m], xo, COPY, scale=ccw)

                    # half-length scans over even positions (chained)
                    i1 = 0.0 if j == 0 else h1_prev
                    i2 = 0.0 if j == 0 else h2_prev
                    h1 = h_pool.tile([d_loc, t_chunk // 2], F32, tag="h1")
                    h2 = h_pool.tile([d_loc, t_chunk // 2], F32, tag="h2")
                    nc.vector.tensor_tensor_scan(
                        h1[:, :m], q1sq_b[:, :m], u1[:, :m], i1, mult, add
                    )
                    nc.vector.tensor_tensor_scan(
                        h2[:, :m], q2sq_b[:, :m], u2[:, :m], i2, mult, add
                    )
                    h1_prev = h1[:, m - 1 : m]
                    h2_prev = h2[:, m - 1 : m]

                    # combines (Vector, fused muladds)
                    ue = acc_pool.tile([d_loc, t_chunk // 2], F32, tag="ue")
                    re = acc_pool.tile([d_loc, t_chunk // 2], F32, tag="re")
                    nc.vector.scalar_tensor_tensor(
                        ue[:, :m], h1[:, :m], c1, t1e[:, :m], mult, add
                    )
                    nc.vector.scalar_tensor_tensor(
                        re[:, :m], h2[:, :m], c2, ue[:, :m], mult, add
                    )
                    uo = acc_pool.tile([d_loc, t_chunk // 2], F32, tag="uo")
                    ro = acc_pool.tile([d_loc, t_chunk // 2], F32, tag="ro")
                    nc.vector.scalar_tensor_tensor(
                        uo[:, :m], h1[:, :m], c1q1, t1o[:, :m], mult, add
                    )
                    nc.vector.scalar_tensor_tensor(
                        ro[:, :m], h2[:, :m], c2q2, uo[:, :m], mult, add
                    )

                    # silu with interleaving strided writes (Scalar)
                    ot = tmp_pool.tile([d_loc, t_chunk], F32, tag="ot")
                    ot2 = ot[:, : 2 * m].rearrange("p (m two) -> p m two", two=2)
                    nc.scalar.activation(ot2[:, :, 0], re[:, :m], ACT)
                    nc.scalar.activation(ot2[:, :, 1], ro[:, :m], ACT)
                    nc.sync.dma_start(
                        out=o_d[bi, :, t0 : t0 + tc_len], in_=ot[:, :tc_len]
                    )
                    t0 += tc_len
                assert t0 == s

    nc.compile()
    return nc


def _host_params(delta, alpha, beta, gamma, omega):
    """Compute per-channel scan params on the host (O(D*N) work)."""
    p = 1.0 / (1.0 + np.exp(-delta[:, :, 0].astype(np.float64)))  # [D, N]
    a = 1.0 / (1.0 + np.exp(-alpha[:, :, 0].astype(np.float64)))
    q = 1.0 - p * a                                               # [D, N]
    c = p * beta[:, :, 0].astype(np.float64) * gamma.astype(np.float64) * SCALE
    pp = np.zeros((D, 12), dtype=np.float32)
    pp[:, 0] = q[:, 0]
    pp[:, 1] = q[:, 1]
    pp[:, 2] = c[:, 0]
    pp[:, 3] = c[:, 1]
    pp[:, 4] = omega
    pp[:, 5] = q[:, 0] ** 2
    pp[:, 6] = q[:, 1] ** 2
    pp[:, 7] = c[:, 0] * q[:, 0]
    pp[:, 8] = c[:, 1] * q[:, 1]
    pp[:, 9] = c[:, 0] + c[:, 1] + omega
    return pp


_NC_CACHE = {}


def kernel(x, delta, alpha, beta, gamma, omega):
    x = np.asarray(x, dtype=np.float32)
    delta = np.asarray(delta, dtype=np.float32)
    alpha = np.asarray(alpha, dtype=np.float32)
    beta = np.asarray(beta, dtype=np.float32)
    gamma = np.asarray(gamma, dtype=np.float32)
    omega = np.asarray(omega, dtype=np.float32)
    assert x.shape == (B, S, D)

    if "nc" not in _NC_CACHE:
        _NC_CACHE["nc"] = build_nc(
            t_chunk=4096, x_bufs=2, h_bufs=2, tmp_bufs=2, acc_bufs=1
        )
    nc = _NC_CACHE["nc"]

    pp = _host_params(delta, alpha, beta, gamma, omega)
    xt = np.ascontiguousarray(x.transpose(0, 2, 1))  # [B, D, S]

    in_maps = []
    for i in range(N_CORES):
        sl = slice(i * D_LOC, (i + 1) * D_LOC)
        in_maps.append(
            {
                "x": np.ascontiguousarray(xt[:, sl, :]),
                "pp": np.ascontiguousarray(pp[sl]),
            }
        )

    res = run_bass_kernel_spmd(nc, in_maps, core_ids=list(range(N_CORES)))

    out = np.empty((B, S, D), dtype=np.float32)
    for i in range(N_CORES):
        sl = slice(i * D_LOC, (i + 1) * D_LOC)
        out[:, :, sl] = res.results[i]["o"].transpose(0, 2, 1)
    return out


# revision 27
# speedup vs baseline: 1.0025x; 1.0025x over previous
"""MultiHeadEMA Trainium2 Bass kernel.

Reference computation (B=4, S=8192, D=1024, N=2):
    out = silu(conv_causal(x, k) + x * omega)
    k[d, l] = sum_n c[d, n] * q[d, n]^l
    q = 1 - sigmoid(delta) * sigmoid(alpha)
    c = sigmoid(delta) * beta * gamma * sqrt(1/N)

The length-S causal conv with a sum-of-2-exponentials kernel is a pair of
first-order linear recurrences (EMA scans):
    h_n[t] = q_n * h_n[t-1] + x[t]
    y[t]   = c_1 h_1[t] + c_2 h_2[t]
    out[t] = silu(y[t] + omega * x[t])

Sharding: D=1024 split across 8 cores (128 channels each).  Each core works
in [channel-partition, time-free] layout; the scans run on the Vector engine
via TensorTensorScanArith, one recurrence per partition.  The host transposes
x to [B, D, S] while slicing the per-core shards and transposes the per-core
results back while gathering (part of the shard/unshard contract).
"""

import math

import numpy as np

import concourse.bass as bass
import concourse.mybir as mybir
import concourse.tile as tile
from concourse import bacc
from concourse.bass_utils import run_bass_kernel_spmd

B = 4
S = 8192
D = 1024
N_CORES = 8
D_LOC = D // N_CORES  # 128 channels per core
SCALE = math.sqrt(1.0 / 2.0)

F32 = mybir.dt.float32


def build_nc(b=B, d_loc=D_LOC, s=S, t_chunk=2048, act="Silu",
             x_bufs=3, h_bufs=3, tmp_bufs=3, acc_bufs=2):
    """Build the per-core Bass module (SPMD: same NEFF on all cores).

    Inputs (per core):
      x  [b, d_loc, s] f32 — time-major-last shard of the input
      pp [d_loc, 8]    f32 — packed params: q1 q2 c1 c2 w (cols 0-4)
    Output:
      o  [b, d_loc, s] f32
    """
    assert s % t_chunk == 0
    n_chunks = s // t_chunk
    # Non-uniform chunk schedule: small chunks at the very start (fill the
    # pipeline quickly) and at the very end (short drain tail).  Middle runs
    # at full t_chunk.  Only the first/last batch get the ramps.
    def chunk_schedule(bi):
        full = [t_chunk] * n_chunks
        ramp = [t_chunk // 8, t_chunk // 8, t_chunk // 4, t_chunk // 2]
        if bi == 0 and n_chunks >= 2:
            return ramp + [t_chunk] * (n_chunks - 1)
        if bi == b - 1 and n_chunks >= 2:
            return [t_chunk] * (n_chunks - 1) + ramp[::-1]
        return full

    nc = bacc.Bacc(
        "TRN2",
        target_bir_lowering=False,
        debug=False,
        enable_asserts=False,
        num_devices=N_CORES,
    )

    x_d = nc.dram_tensor("x", [b, d_loc, s], F32, kind="ExternalInput").ap()
    pp_d = nc.dram_tensor("pp", [d_loc, 12], F32, kind="ExternalInput").ap()
    o_d = nc.dram_tensor("o", [b, d_loc, s], F32, kind="ExternalOutput").ap()

    with tile.TileContext(nc) as tc:
        with (
            tc.tile_pool(name="pp", bufs=1) as pp_pool,
            tc.tile_pool(name="x", bufs=x_bufs) as x_pool,
            tc.tile_pool(name="h", bufs=h_bufs) as h_pool,
            tc.tile_pool(name="tmp", bufs=tmp_bufs) as tmp_pool,
            tc.tile_pool(name="acc", bufs=acc_bufs) as acc_pool,
        ):
            # pp rides the GpSimd SWDGE path so the HWDGE queue's first
            # (cold, ~3us setup) transfer is the first x chunk itself —
            # overlapping the two queue spin-ups at kernel start.
            pp = pp_pool.tile([d_loc, 12], F32, tag="pp")
            nc.gpsimd.dma_start(out=pp[:], in_=pp_d[:])
            q1 = pp[:, 0:1]
            q2 = pp[:, 1:2]
            c1 = pp[:, 2:3]
            c2 = pp[:, 3:4]
            w = pp[:, 4:5]
            q1sq_b = pp[:, 5:6].broadcast_to([d_loc, t_chunk // 2])
            q2sq_b = pp[:, 6:7].broadcast_to([d_loc, t_chunk // 2])
            c1q1 = pp[:, 7:8]
            c2q2 = pp[:, 8:9]
            ccw = pp[:, 9:10]

            mult = mybir.AluOpType.mult
            add = mybir.AluOpType.add
            COPY = mybir.ActivationFunctionType.Copy
            ACT = getattr(mybir.ActivationFunctionType, act)

            # Radix-2 polyphase: the time-major scan halves its length by
            # scanning only even positions (h_e[m] = q^2 h_e[m-1] + u[m],
            # u[m] = q*x[2m-1] + x[2m]); odd positions never materialize —
            # they fold into the combine as r_odd = c1q1*h1e + c2q2*h2e +
            # (c1+c2+w)*x_odd.  Strided SBUF access is full-rate on both
            # DVE and ACT (measured), so only the scan shrinks.
            h1_prev = None
            h2_prev = None
            for bi in range(b):
                t0 = 0
                for j, tc_len in enumerate(chunk_schedule(bi)):
                    m = tc_len // 2
                    # x tile with 1-element halo in column 0 (= x[t0-1])
                    xt = x_pool.tile([d_loc, t_chunk + 2], F32, tag="x")
                    if j == 0:
                        nc.vector.memset(xt[:, 0:1], 0.0)
                        nc.sync.dma_start(
                            out=xt[:, 1 : tc_len + 1],
                            in_=x_d[bi, :, t0 : t0 + tc_len],
                        )
                    else:
                        nc.sync.dma_start(
                            out=xt[:, 0 : tc_len + 1],
                            in_=x_d[bi, :, t0 - 1 : t0 + tc_len],
                        )
                    # phase views (columns: [halo, x0, x1, ..., x_{T-1}, pad])
                    xop = xt[:, 0 : 2 * m].rearrange("p (m two) -> p m two", two=2)[:, :, 0]   # x[2m-1]
                    xe = xt[:, 1 : 2 * m + 1].rearrange("p (m two) -> p m two", two=2)[:, :, 0]  # x[2m]
                    xo = xt[:, 2 : 2 * m + 2].rearrange("p (m two) -> p m two", two=2)[:, :, 0]  # x[2m+1]

                    # u_n = q_n * x[2m-1] + x[2m]    (Vector)
                    u1 = acc_pool.tile([d_loc, t_chunk // 2], F32, tag="u1")
                    u2 = acc_pool.tile([d_loc, t_chunk // 2], F32, tag="u2")
                    nc.vector.scalar_tensor_tensor(u1[:, :m], xop, q1, xe, mult, add)
                    nc.vector.scalar_tensor_tensor(u2[:, :m], xop, q2, xe, mult, add)

                    # residual pre-scales on the Scalar engine
                    t1e = tmp_pool.tile([d_loc, t_chunk // 2], F32, tag="t1e")
                    t1o = tmp_pool.tile([d_loc, t_chunk // 2], F32, tag="t1o")
                    nc.scalar.activation(t1e[:, :m], xe, COPY, scale=w)
                    nc.scalar.activation(t1o[:, :m], xo, COPY, scale=ccw)

                    # half-length scans over even positions (chained)
                    i1 = 0.0 if j == 0 else h1_prev
                    i2 = 0.0 if j == 0 else h2_prev
                    h1 = h_pool.tile([d_loc, t_chunk // 2], F32, tag="h1")
                    h2 = h_pool.tile([d_loc, t_chunk // 2], F32, tag="h2")
                    nc.vector.tensor_tensor_scan(
                        h1[:, :m], q1sq_b[:, :m], u1[:, :m], i1, mult, add
                    )
                    nc.vector.tensor_tensor_scan(
                        h2[:, :m], q2sq_b[:, :m], u2[:, :m], i2, mult, add
                    )
                    h1_prev = h1[:, m - 1 : m]
                    h2_prev = h2[:, m - 1 : m]

                    # combines (Vector, fused muladds)
                    ue = acc_pool.tile([d_loc, t_chunk // 2], F32, tag="ue")
                    re = acc_pool.tile([d_loc, t_chunk // 2], F32, tag="re")
                    nc.vector.scalar_tensor_tensor(
                        ue[:, :m], h1[:, :m], c1, t1e[:, :m], mult, add
                    )
                    nc.vector.scalar_tensor_tensor(
                        re[:, :m], h2[:, :m], c2, ue[:, :m], mult, add
                    )
                    uo = acc_pool.tile([d_loc, t_chunk // 2], F32, tag="uo")
                    ro = acc_pool.tile([d_loc, t_chunk // 2], F32, tag="ro")
                    nc.vector.scalar_tensor_tensor(
                        uo[:, :m], h1[:, :m], c1q1, t1o[:, :m], mult, add
                    )
                    nc.vector.scalar_tensor_tensor(
                        ro[:, :m], h2[:, :m], c2q2, uo[:, :m], mult, add
                    )

                    # silu with interleaving strided writes (Scalar)
                    ot = tmp_pool.tile([d_loc, t_chunk], F32, tag="ot")
                    ot2 = ot[:, : 2 * m].rearrange("p (m two) -> p m two", two=2)
                    nc.scalar.activation(ot2[:, :, 0], re[:, :m], ACT)
                    nc.scalar.activation(ot2[:, :, 1], ro[:, :m], ACT)
                    nc.sync.dma_start(
                        out=o_d[bi, :, t0 : t0 + tc_len], in_=ot[:, :tc_len]
                    )
                    t0 += tc_len
                assert t0 == s

    nc.compile()
    return nc


def _host_params(delta, alpha, beta, gamma, omega):
    """Compute per-channel scan params on the host (O(D*N) work)."""
    p = 1.0 / (1.0 + np.exp(-delta[:, :, 0].astype(np.float64)))  # [D, N]
    a = 1.0 / (1.0 + np.exp(-alpha[:, :, 0].astype(np.float64)))
    q = 1.0 - p * a                                               # [D, N]
    c = p * beta[:, :, 0].astype(np.float64) * gamma.astype(np.float64) * SCALE
    pp = np.zeros((D, 12), dtype=np.float32)
    pp[:, 0] = q[:, 0]
    pp[:, 1] = q[:, 1]
    pp[:, 2] = c[:, 0]
    pp[:, 3] = c[:, 1]
    pp[:, 4] = omega
    pp[:, 5] = q[:, 0] ** 2
    pp[:, 6] = q[:, 1] ** 2
    pp[:, 7] = c[:, 0] * q[:, 0]
    pp[:, 8] = c[:, 1] * q[:, 1]
    pp[:, 9] = c[:, 0] + c[:, 1] + omega
    return pp


_NC_CACHE = {}


def kernel(x, delta, alpha, beta, gamma, omega):
    x = np.asarray(x, dtype=np.float32)
    delta = np.asarray(delta, dtype=np.float32)
    alpha = np.asarray(alpha, dtype=np.float32)
    beta = np.asarray(beta, dtype=np.float32)
    gamma = np.asarray(gamma, dtype=np.float32)
    omega = np.asarray(omega, dtype=np.float32)
    assert x.shape == (B, S, D)

    if "nc" not in _NC_CACHE:
        _NC_CACHE["nc"] = build_nc(
            t_chunk=4096, x_bufs=2, h_bufs=2, tmp_bufs=2, acc_bufs=1
        )
    nc = _NC_CACHE["nc"]

    pp = _host_params(delta, alpha, beta, gamma, omega)
    xt = np.ascontiguousarray(x.transpose(0, 2, 1))  # [B, D, S]

    in_maps = []
    for i in range(N_CORES):
        sl = slice(i * D_LOC, (i + 1) * D_LOC)
        in_maps.append(
            {
                "x": np.ascontiguousarray(xt[:, sl, :]),
                "pp": np.ascontiguousarray(pp[sl]),
            }
        )

    res = run_bass_kernel_spmd(nc, in_maps, core_ids=list(range(N_CORES)))

    out = np.empty((B, S, D), dtype=np.float32)
    for i in range(N_CORES):
        sl = slice(i * D_LOC, (i + 1) * D_LOC)
        out[:, :, sl] = res.results[i]["o"].transpose(0, 2, 1)
    return out
